# revision 7
# baseline (speedup 1.0000x reference)
"""Trainium2 Bass kernel for 24-rotation (octahedral) 3D conv (ConvZ3P24).

Problem: x (2,4,64,64,64) f32, weight (8,4,3,3,3), bias (8,)
  -> y (2,24,8,64,64,64):  conv3d(x, rotated_filter_bank) + bias,
  stride 1, pad 1, 24 proper octahedral rotations x 8 cout = 192 channels.

Sharding: 8 cores = batch(2) x depth-chunks(4 x 16 planes). Each core
computes all 192 channels for its 16 output planes (50.3 MB out).

Device kernel (per core):
  - im2col "ring": per padded input plane s (18 per core), one DMA builds an
    SBUF tile [36, 4224] whose partition p=(kh,kw,ci) holds the plane
    shifted by (kh,kw) within the zero-padded 66x66 frame.
  - output plane d, tile (mhalf, hblock): PSUM [96,512] accumulates 3
    float32r matmuls (kd=0,1,2) with lhsT = rotated-weight slices [36,96]
    and rhs = ring tile d+kd viewed [36, 8, 64] at h-block offset.
  - PSUM evacuated to SBUF with fused per-channel bias add (DVE for
    channel half 0, ACT for half 1), staged to [96, 4096] per plane, then
    DMA'd to the per-core output (192,16,64,64).
"""

import itertools
from contextlib import ExitStack

import numpy as np

# ---------------------------------------------------------------- constants
CIN = 4
COUT = 8
N_ROT = 24
KS = 3
DHW = 64
PH = 66            # padded plane side
PLANE = PH * PH    # 4356
SLOT = 4224        # ring slot floats per partition (= 64*66)
VALID = 4222       # floats actually written per partition (max read idx 4221)
N_CORES = 8
DCHUNK = 16        # output planes per core
SLAB_D = 18        # input padded planes per core (16 + 2 halo)
RING = 6
M = 192            # total output channels (24 rot * 8 cout)
MHALF = 96
NTAP = 36          # (kh,kw,ci) rows per kd
HB = 8             # h-blocks per plane
NCOL = 512         # matmul free dim (8 h-rows * 64)

_CACHE = {}


def _rot_index_maps(k):
    """Source voxel indices (d,h,w) for the 24 proper octahedral rotations."""
    m = (k - 1) // 2
    mats = []
    for perm in itertools.permutations(range(3)):
        for signs in itertools.product([1, -1], repeat=3):
            R = np.zeros((3, 3))
            for i in range(3):
                R[i, perm[i]] = signs[i]
            if np.linalg.det(R) > 0.5:
                mats.append(R)
    c = np.arange(k) - m
    cz, cy, cx = np.meshgrid(c, c, c, indexing="ij")
    v = np.stack([cx, cy, cz], 0).astype(np.float64)
    idx = []
    for R in mats:
        g = np.einsum("ij,jdhw->idhw", R, v)
        idx.append(
            np.stack([g[2] + m, g[1] + m, g[0] + m], 0).round().astype(np.int64)
        )
    return np.stack(idx, 0)  # (24, 3, k, k, k)


def _build_program():
    import concourse.bacc as bacc
    import concourse.bass as bass
    import concourse.mybir as mybir
    import concourse.tile as tile

    f32 = mybir.dt.float32
    f32r = mybir.dt.float32r

    nc = bacc.Bacc(
        "TRN2",
        target_bir_lowering=False,
        debug=False,
        enable_asserts=False,
        num_devices=N_CORES,
    )

    xs = nc.dram_tensor("xs", (CIN, SLAB_D, PH, PH), f32r, kind="ExternalInput")
    wt = nc.dram_tensor("wt", (KS, NTAP, M), f32r, kind="ExternalInput")
    bias2 = nc.dram_tensor("bias2", (MHALF, 2), f32, kind="ExternalInput")
    y = nc.dram_tensor("y", (M, DCHUNK, DHW, DHW), f32, kind="ExternalOutput")
    xs_ap = xs.ap()
    y_ap = y.ap()

    ci_stride = SLAB_D * PLANE  # elements between ci channels in xs

    with tile.TileContext(nc) as tc:
        with ExitStack() as ctx:
            wpool = ctx.enter_context(tc.tile_pool(name="wpool", bufs=1))
            ring = ctx.enter_context(tc.tile_pool(name="ring", bufs=RING))
            psum = ctx.enter_context(
                tc.tile_pool(name="psum", bufs=8, space="PSUM")
            )
            stage = ctx.enter_context(tc.tile_pool(name="stage", bufs=4))

            # ---- weights + bias (tiny, once)
            w_t = wpool.tile([NTAP, KS * M], f32r)
            for kd in range(KS):
                nc.sync.dma_start(
                    w_t[:, kd * M : (kd + 1) * M], wt.ap()[kd]
                )
            bias_t = wpool.tile([MHALF, 2], f32)
            nc.sync.dma_start(bias_t[:], bias2.ap())

            handles = {}

            def load_plane(s):
                t = ring.tile([NTAP, SLOT], f32r, tag="ring", name=f"ring{s}")
                for kh in range(KS):
                    src = bass.AP(
                        xs_ap.tensor,
                        s * PLANE + kh * PH,
                        [[1, KS], [ci_stride, CIN], [1, VALID]],
                    )
                    nc.sync.dma_start(
                        t[kh * 12 : (kh + 1) * 12, 0:VALID], src
                    )
                handles[s] = t

            load_plane(0)
            load_plane(1)

            for d in range(DCHUNK):
                load_plane(d + 2)
                rhs_v = [
                    handles[d + kd][:].rearrange("p (h w) -> p h w", w=PH)
                    for kd in range(KS)
                ]
                for mh in range(2):
                    st = stage.tile([MHALF, HB * NCOL], f32, tag="stage")
                    ps_tiles = []
                    for hb in range(HB):
                        ps = psum.tile([MHALF, NCOL], f32, tag="ps")
                        ps_tiles.append(ps)
                    for kd in range(KS):
                        lhsT = w_t[:, kd * M + mh * MHALF : kd * M + (mh + 1) * MHALF]
                        for hb in range(HB):
                            rhs = rhs_v[kd][:, hb * HB : hb * HB + HB, 0:DHW]
                            nc.tensor.matmul(
                                ps_tiles[hb][:],
                                lhsT,
                                rhs,
                                start=(kd == 0),
                                stop=(kd == KS - 1),
                            )
                    for hb in range(HB):
                        dst = st[:, hb * NCOL : (hb + 1) * NCOL]
                        if mh == 0:
                            nc.vector.tensor_scalar_add(
                                dst, ps_tiles[hb][:], bias_t[:, 0:1]
                            )
                        else:
                            nc.scalar.activation(
                                dst,
                                ps_tiles[hb][:],
                                mybir.ActivationFunctionType.Identity,
                                bias=bias_t[:, 1:2],
                            )
                    nc.sync.dma_start(
                        y_ap[mh * MHALF : (mh + 1) * MHALF, d], st[:]
                    )

    nc.compile()
    return nc


def _make_runner(nc):
    """Build a reusable jitted SPMD executor (no donation so device buffers
    can be reused across timing calls). Modeled on bass2jax.run_bass_via_pjrt."""
    import jax
    import numpy as _np
    from jax.sharding import Mesh, PartitionSpec
    from jax.experimental.shard_map import shard_map

    import concourse.mybir as mybir
    from concourse import bass2jax

    bass2jax.install_neuronx_cc_hook()

    partition_name = (
        nc.partition_id_tensor.name if nc.partition_id_tensor else None
    )
    in_names, out_names, out_avals, zero_outs = [], [], [], []
    for alloc in nc.m.functions[0].allocations:
        if not isinstance(alloc, mybir.MemoryLocationSet):
            continue
        name = alloc.memorylocations[0].name
        if alloc.kind == "ExternalInput":
            if name != partition_name:
                in_names.append(name)
        elif alloc.kind == "ExternalOutput":
            shape = tuple(alloc.tensor_shape)
            dtype = mybir.dt.np(alloc.dtype)
            out_names.append(name)
            out_avals.append(jax.core.ShapedArray(shape, dtype))
            zero_outs.append(_np.zeros(shape, dtype))
    n_params = len(in_names)
    all_names = in_names + out_names
    if partition_name is not None:
        all_names = all_names + [partition_name]

    def _body(*args):
        operands = list(args)
        if partition_name is not None:
            operands.append(bass2jax.partition_id_tensor())
        outs = bass2jax._bass_exec_p.bind(
            *operands,
            out_avals=tuple(out_avals),
            in_names=tuple(all_names),
            out_names=tuple(out_names),
            lowering_input_output_aliases=(),
            sim_require_finite=True,
            sim_require_nnan=True,
            nc=nc,
        )
        return tuple(outs)

    devices = jax.devices()[:N_CORES]
    mesh = Mesh(np.asarray(devices), ("core",))
    n_args = n_params + len(out_names)
    sharded = jax.jit(
        shard_map(
            _body,
            mesh=mesh,
            in_specs=(PartitionSpec("core"),) * n_args,
            out_specs=(PartitionSpec("core"),) * len(out_names),
            check_rep=False,
        ),
        keep_unused=True,
    )

    from jax.sharding import NamedSharding

    shard = NamedSharding(mesh, PartitionSpec("core"))

    def place_inputs(in_maps):
        """Device-put per-core inputs (sharded along axis 0) + cached zero
        output buffers; returns the full arg list, all device-resident."""
        concat = [
            np.concatenate([np.asarray(m[name]) for m in in_maps], axis=0)
            for name in in_names
        ]
        placed = [jax.device_put(a, shard) for a in concat]
        if "zeros" not in _CACHE:
            _CACHE["zeros"] = [
                jax.device_put(
                    np.zeros((N_CORES * z.shape[0], *z.shape[1:]), z.dtype), shard
                )
                for z in zero_outs
            ]
        return placed + _CACHE["zeros"]

    def run(args):
        return sharded(*args)

    return {
        "place_inputs": place_inputs,
        "run": run,
        "out_names": out_names,
        "out_avals": out_avals,
    }


def _get_runner():
    if "runner" not in _CACHE:
        nc = _build_program()
        _CACHE["runner"] = _make_runner(nc)
    return _CACHE["runner"]


def _host_prep(x, weight, bias):
    idx = _rot_index_maps(KS)
    wr = weight[:, :, idx[:, 0], idx[:, 1], idx[:, 2]]  # (8,4,24,3,3,3)
    wr = np.transpose(wr, (2, 0, 1, 3, 4, 5)).reshape(M, CIN, KS, KS, KS)
    # lhsT rows ordered (kh, kw, ci) to match the im2col partition order.
    wt = np.ascontiguousarray(
        wr.transpose(2, 3, 4, 1, 0).reshape(KS, NTAP, M), dtype=np.float32
    )
    bias192 = np.broadcast_to(bias[None, :], (N_ROT, COUT)).reshape(M)
    bias2 = np.ascontiguousarray(bias192.reshape(2, MHALF).T, dtype=np.float32)

    x_pad = np.zeros((2, CIN, PH, PH, PH), dtype=np.float32)
    x_pad[:, :, 1:65, 1:65, 1:65] = x

    in_maps = []
    for core in range(N_CORES):
        n, dc = divmod(core, N_CORES // 2)
        slab = np.ascontiguousarray(
            x_pad[n, :, DCHUNK * dc : DCHUNK * dc + SLAB_D]
        )
        in_maps.append({"xs": slab, "wt": wt, "bias2": bias2})
    return in_maps


def kernel(x, weight, bias):
    x = np.asarray(x, dtype=np.float32)
    weight = np.asarray(weight, dtype=np.float32)
    bias = np.asarray(bias, dtype=np.float32)

    runner = _get_runner()
    in_maps = _host_prep(x, weight, bias)
    args = runner["place_inputs"](in_maps)
    out = runner["run"](args)
    y8 = np.asarray(out[0]).reshape(N_CORES, M, DCHUNK, DHW, DHW)

    yfull = np.empty((2, M, DHW, DHW, DHW), dtype=np.float32)
    for core in range(N_CORES):
        n, dc = divmod(core, N_CORES // 2)
        yfull[n, :, DCHUNK * dc : DCHUNK * (dc + 1)] = y8[core]
    return yfull.reshape(2, N_ROT, COUT, DHW, DHW, DHW)


# revision 11
# speedup vs baseline: 170.3143x; 170.3143x over previous
"""Trainium2 Bass kernel for 24-rotation (octahedral) 3D conv (ConvZ3P24).

Problem: x (2,4,64,64,64) f32, weight (8,4,3,3,3), bias (8,)
  -> y (2,24,8,64,64,64):  conv3d(x, rotated_filter_bank) + bias,
  stride 1, pad 1, 24 proper octahedral rotations x 8 cout = 192 channels.

Sharding: 8 cores = batch(2) x depth-chunks(4 x 16 planes). Each core
computes all 192 channels for its 16 output planes (50.3 MB out).

Device kernel (per core):
  - im2col "ring": per padded input plane s (18 per core), one DMA builds an
    SBUF tile [36, 4224] whose partition p=(kh,kw,ci) holds the plane
    shifted by (kh,kw) within the zero-padded 66x66 frame.
  - output plane d, tile (mhalf, hblock): PSUM [96,512] accumulates 3
    float32r matmuls (kd=0,1,2) with lhsT = rotated-weight slices [36,96]
    and rhs = ring tile d+kd viewed [36, 8, 64] at h-block offset.
  - PSUM evacuated to SBUF with fused per-channel bias add (DVE for
    channel half 0, ACT for half 1), staged to [96, 4096] per plane, then
    DMA'd to the per-core output (192,16,64,64).
"""

import itertools
from contextlib import ExitStack

import numpy as np

# ---------------------------------------------------------------- constants
CIN = 4
COUT = 8
N_ROT = 24
KS = 3
DHW = 64
PH = 66            # padded plane side
PLANE = PH * PH    # 4356
SLOT = 4224        # ring slot floats per partition (= 64*66)
VALID = 4222       # floats actually written per partition (max read idx 4221)
N_CORES = 8
DCHUNK = 16        # output planes per core
SLAB_D = 18        # input padded planes per core (16 + 2 halo)
RING = 6
M = 192            # total output channels (24 rot * 8 cout)
MHALF = 96
NTAP = 36          # (kh,kw,ci) rows per kd
HB = 8             # h-blocks per plane
NCOL = 512         # matmul free dim (8 h-rows * 64)

_CACHE = {}


def _rot_index_maps(k):
    """Source voxel indices (d,h,w) for the 24 proper octahedral rotations."""
    m = (k - 1) // 2
    mats = []
    for perm in itertools.permutations(range(3)):
        for signs in itertools.product([1, -1], repeat=3):
            R = np.zeros((3, 3))
            for i in range(3):
                R[i, perm[i]] = signs[i]
            if np.linalg.det(R) > 0.5:
                mats.append(R)
    c = np.arange(k) - m
    cz, cy, cx = np.meshgrid(c, c, c, indexing="ij")
    v = np.stack([cx, cy, cz], 0).astype(np.float64)
    idx = []
    for R in mats:
        g = np.einsum("ij,jdhw->idhw", R, v)
        idx.append(
            np.stack([g[2] + m, g[1] + m, g[0] + m], 0).round().astype(np.int64)
        )
    return np.stack(idx, 0)  # (24, 3, k, k, k)


def _build_program(repeat=1):
    import concourse.bacc as bacc
    import concourse.bass as bass
    import concourse.mybir as mybir
    import concourse.tile as tile

    f32 = mybir.dt.float32
    f32r = mybir.dt.float32r

    nc = bacc.Bacc(
        "TRN2",
        target_bir_lowering=False,
        debug=False,
        enable_asserts=False,
        num_devices=N_CORES,
    )

    xs = nc.dram_tensor("xs", (CIN, SLAB_D, PH, PH), f32r, kind="ExternalInput")
    wt = nc.dram_tensor("wt", (KS, NTAP, M), f32r, kind="ExternalInput")
    bias2 = nc.dram_tensor("bias2", (MHALF, 2), f32, kind="ExternalInput")
    y = nc.dram_tensor("y", (M, DCHUNK, DHW, DHW), f32, kind="ExternalOutput")
    xs_ap = xs.ap()
    y_ap = y.ap()

    ci_stride = SLAB_D * PLANE  # elements between ci channels in xs

    with tile.TileContext(nc) as tc:
        with ExitStack() as ctx:
            wpool = ctx.enter_context(tc.tile_pool(name="wpool", bufs=1))
            ring = ctx.enter_context(tc.tile_pool(name="ring", bufs=RING))
            psum = ctx.enter_context(
                tc.tile_pool(name="psum", bufs=8, space="PSUM")
            )
            stage = ctx.enter_context(tc.tile_pool(name="stage", bufs=4))

            # ---- weights + bias (tiny, once)
            w_t = wpool.tile([NTAP, KS * M], f32r)
            for kd in range(KS):
                nc.sync.dma_start(
                    w_t[:, kd * M : (kd + 1) * M], wt.ap()[kd]
                )
            bias_t = wpool.tile([MHALF, 2], f32)
            nc.sync.dma_start(bias_t[:], bias2.ap())

            handles = {}
            loop_ctx = ExitStack()
            if repeat > 1:
                loop_ctx.enter_context(
                    tc.For_i(
                        0,
                        repeat,
                        1,
                        hint_engines=(
                            mybir.EngineType.PE,
                            mybir.EngineType.DVE,
                            mybir.EngineType.Activation,
                            mybir.EngineType.SP,
                            mybir.EngineType.Pool,
                        ),
                    )
                )

            def load_plane(s):
                t = ring.tile([NTAP, SLOT], f32r, tag="ring", name=f"ring{s}")
                for kh in range(KS):
                    src = bass.AP(
                        xs_ap.tensor,
                        s * PLANE + kh * PH,
                        [[1, KS], [ci_stride, CIN], [1, VALID]],
                    )
                    nc.sync.dma_start(
                        t[kh * 12 : (kh + 1) * 12, 0:VALID], src
                    )
                handles[s] = t

            load_plane(0)
            load_plane(1)

            for d in range(DCHUNK):
                load_plane(d + 2)
                rhs_v = [
                    handles[d + kd][:].rearrange("p (h w) -> p h w", w=PH)
                    for kd in range(KS)
                ]
                for mh in range(2):
                    st = stage.tile([MHALF, HB * NCOL], f32, tag="stage")
                    ps_tiles = []
                    for hb in range(HB):
                        ps = psum.tile([MHALF, NCOL], f32, tag="ps")
                        ps_tiles.append(ps)
                    for kd in range(KS):
                        lhsT = w_t[:, kd * M + mh * MHALF : kd * M + (mh + 1) * MHALF]
                        for hb in range(HB):
                            rhs = rhs_v[kd][:, hb * HB : hb * HB + HB, 0:DHW]
                            nc.tensor.matmul(
                                ps_tiles[hb][:],
                                lhsT,
                                rhs,
                                start=(kd == 0),
                                stop=(kd == KS - 1),
                            )
                    for hb in range(HB):
                        dst = st[:, hb * NCOL : (hb + 1) * NCOL]
                        if mh == 0:
                            nc.vector.tensor_scalar_add(
                                dst, ps_tiles[hb][:], bias_t[:, 0:1]
                            )
                        else:
                            nc.scalar.activation(
                                dst,
                                ps_tiles[hb][:],
                                mybir.ActivationFunctionType.Identity,
                                bias=bias_t[:, 1:2],
                            )
                    nc.sync.dma_start(
                        y_ap[mh * MHALF : (mh + 1) * MHALF, d], st[:]
                    )

            loop_ctx.close()

    nc.compile()
    return nc


def _make_runner(nc):
    """Build a reusable jitted SPMD executor (no donation so device buffers
    can be reused across timing calls). Modeled on bass2jax.run_bass_via_pjrt."""
    import jax
    import numpy as _np
    from jax.sharding import Mesh, PartitionSpec
    from jax.experimental.shard_map import shard_map

    import concourse.mybir as mybir
    from concourse import bass2jax

    bass2jax.install_neuronx_cc_hook()

    partition_name = (
        nc.partition_id_tensor.name if nc.partition_id_tensor else None
    )
    in_names, out_names, out_avals, zero_outs = [], [], [], []
    for alloc in nc.m.functions[0].allocations:
        if not isinstance(alloc, mybir.MemoryLocationSet):
            continue
        name = alloc.memorylocations[0].name
        if alloc.kind == "ExternalInput":
            if name != partition_name:
                in_names.append(name)
        elif alloc.kind == "ExternalOutput":
            shape = tuple(alloc.tensor_shape)
            dtype = mybir.dt.np(alloc.dtype)
            out_names.append(name)
            out_avals.append(jax.core.ShapedArray(shape, dtype))
            zero_outs.append(_np.zeros(shape, dtype))
    n_params = len(in_names)
    all_names = in_names + out_names
    if partition_name is not None:
        all_names = all_names + [partition_name]

    def _body(*args):
        operands = list(args)
        if partition_name is not None:
            operands.append(bass2jax.partition_id_tensor())
        outs = bass2jax._bass_exec_p.bind(
            *operands,
            out_avals=tuple(out_avals),
            in_names=tuple(all_names),
            out_names=tuple(out_names),
            lowering_input_output_aliases=(),
            sim_require_finite=True,
            sim_require_nnan=True,
            nc=nc,
        )
        return tuple(outs)

    devices = jax.devices()[:N_CORES]
    mesh = Mesh(np.asarray(devices), ("core",))
    n_args = n_params + len(out_names)
    sharded = jax.jit(
        shard_map(
            _body,
            mesh=mesh,
            in_specs=(PartitionSpec("core"),) * n_args,
            out_specs=(PartitionSpec("core"),) * len(out_names),
            check_rep=False,
        ),
        keep_unused=True,
    )

    from jax.sharding import NamedSharding

    shard = NamedSharding(mesh, PartitionSpec("core"))

    def place_inputs(in_maps):
        """Device-put per-core inputs (sharded along axis 0) + cached zero
        output buffers; returns the full arg list, all device-resident."""
        concat = [
            np.concatenate([np.asarray(m[name]) for m in in_maps], axis=0)
            for name in in_names
        ]
        placed = [jax.device_put(a, shard) for a in concat]
        if "zeros" not in _CACHE:
            _CACHE["zeros"] = [
                jax.device_put(
                    np.zeros((N_CORES * z.shape[0], *z.shape[1:]), z.dtype), shard
                )
                for z in zero_outs
            ]
        return placed + _CACHE["zeros"]

    def run(args):
        return sharded(*args)

    return {
        "place_inputs": place_inputs,
        "run": run,
        "out_names": out_names,
        "out_avals": out_avals,
    }


def _get_runner():
    if "runner" not in _CACHE:
        nc = _build_program()
        _CACHE["runner"] = _make_runner(nc)
    return _CACHE["runner"]


def _get_timing_runner(repeat):
    key = f"runner_r{repeat}"
    if key not in _CACHE:
        nc = _build_program(repeat=repeat)
        _CACHE[key] = _make_runner(nc)
    return _CACHE[key]


def _host_prep(x, weight, bias):
    idx = _rot_index_maps(KS)
    wr = weight[:, :, idx[:, 0], idx[:, 1], idx[:, 2]]  # (8,4,24,3,3,3)
    wr = np.transpose(wr, (2, 0, 1, 3, 4, 5)).reshape(M, CIN, KS, KS, KS)
    # lhsT rows ordered (kh, kw, ci) to match the im2col partition order.
    wt = np.ascontiguousarray(
        wr.transpose(2, 3, 4, 1, 0).reshape(KS, NTAP, M), dtype=np.float32
    )
    bias192 = np.broadcast_to(bias[None, :], (N_ROT, COUT)).reshape(M)
    bias2 = np.ascontiguousarray(bias192.reshape(2, MHALF).T, dtype=np.float32)

    x_pad = np.zeros((2, CIN, PH, PH, PH), dtype=np.float32)
    x_pad[:, :, 1:65, 1:65, 1:65] = x

    in_maps = []
    for core in range(N_CORES):
        n, dc = divmod(core, N_CORES // 2)
        slab = np.ascontiguousarray(
            x_pad[n, :, DCHUNK * dc : DCHUNK * dc + SLAB_D]
        )
        in_maps.append({"xs": slab, "wt": wt, "bias2": bias2})
    return in_maps


def kernel(x, weight, bias):
    x = np.asarray(x, dtype=np.float32)
    weight = np.asarray(weight, dtype=np.float32)
    bias = np.asarray(bias, dtype=np.float32)

    runner = _get_runner()
    in_maps = _host_prep(x, weight, bias)
    args = runner["place_inputs"](in_maps)
    out = runner["run"](args)
    y8 = np.asarray(out[0]).reshape(N_CORES, M, DCHUNK, DHW, DHW)

    yfull = np.empty((2, M, DHW, DHW, DHW), dtype=np.float32)
    for core in range(N_CORES):
        n, dc = divmod(core, N_CORES // 2)
        yfull[n, :, DCHUNK * dc : DCHUNK * (dc + 1)] = y8[core]
    return yfull.reshape(2, N_ROT, COUT, DHW, DHW, DHW)


# revision 13
# speedup vs baseline: 290.7791x; 1.7073x over previous
"""Trainium2 Bass kernel for 24-rotation (octahedral) 3D conv (ConvZ3P24).

Problem: x (2,4,64,64,64) f32, weight (8,4,3,3,3), bias (8,)
  -> y (2,24,8,64,64,64):  conv3d(x, rotated_filter_bank) + bias,
  stride 1, pad 1, 24 proper octahedral rotations x 8 cout = 192 channels.

Sharding: 8 cores = batch(2) x depth-chunks(4 x 16 planes). Each core
computes all 192 channels for its 16 output planes (50.3 MB out).

Device kernel (per core):
  - im2col "ring": per padded input plane s (18 per core), one DMA builds an
    SBUF tile [36, 4224] whose partition p=(kh,kw,ci) holds the plane
    shifted by (kh,kw) within the zero-padded 66x66 frame.
  - output plane d, tile (mhalf, hblock): PSUM [96,512] accumulates 3
    float32r matmuls (kd=0,1,2) with lhsT = rotated-weight slices [36,96]
    and rhs = ring tile d+kd viewed [36, 8, 64] at h-block offset.
  - PSUM evacuated to SBUF with fused per-channel bias add (DVE for
    channel half 0, ACT for half 1), staged to [96, 4096] per plane, then
    DMA'd to the per-core output (192,16,64,64).
"""

import itertools
from contextlib import ExitStack

import numpy as np

# ---------------------------------------------------------------- constants
CIN = 4
COUT = 8
N_ROT = 24
KS = 3
DHW = 64
PH = 66            # padded plane side
PLANE = PH * PH    # 4356
SLOT = 4224        # ring slot floats per partition (= 64*66)
VALID = 4222       # floats actually written per partition (max read idx 4221)
N_CORES = 8
DCHUNK = 16        # output planes per core
SLAB_D = 18        # input padded planes per core (16 + 2 halo)
RING = 6
M = 192            # total output channels (24 rot * 8 cout)
MHALF = 96
NTAP = 36          # (kh,kw,ci) rows per kd
HB = 8             # h-blocks per plane
NCOL = 512         # matmul free dim (8 h-rows * 64)

_CACHE = {}


def _rot_index_maps(k):
    """Source voxel indices (d,h,w) for the 24 proper octahedral rotations."""
    m = (k - 1) // 2
    mats = []
    for perm in itertools.permutations(range(3)):
        for signs in itertools.product([1, -1], repeat=3):
            R = np.zeros((3, 3))
            for i in range(3):
                R[i, perm[i]] = signs[i]
            if np.linalg.det(R) > 0.5:
                mats.append(R)
    c = np.arange(k) - m
    cz, cy, cx = np.meshgrid(c, c, c, indexing="ij")
    v = np.stack([cx, cy, cz], 0).astype(np.float64)
    idx = []
    for R in mats:
        g = np.einsum("ij,jdhw->idhw", R, v)
        idx.append(
            np.stack([g[2] + m, g[1] + m, g[0] + m], 0).round().astype(np.int64)
        )
    return np.stack(idx, 0)  # (24, 3, k, k, k)


def _build_program(repeat=1):
    import concourse.bacc as bacc
    import concourse.bass as bass
    import concourse.mybir as mybir
    import concourse.tile as tile

    f32 = mybir.dt.float32
    f32r = mybir.dt.float32r

    nc = bacc.Bacc(
        "TRN2",
        target_bir_lowering=False,
        debug=False,
        enable_asserts=False,
        num_devices=N_CORES,
    )

    xs = nc.dram_tensor("xs", (CIN, SLAB_D, PH, PH), f32r, kind="ExternalInput")
    wt = nc.dram_tensor("wt", (KS, NTAP, M), f32r, kind="ExternalInput")
    bias2 = nc.dram_tensor("bias2", (MHALF, 2), f32, kind="ExternalInput")
    y = nc.dram_tensor("y", (M, DCHUNK, DHW, DHW), f32, kind="ExternalOutput")
    xs_ap = xs.ap()
    y_ap = y.ap()

    ci_stride = SLAB_D * PLANE  # elements between ci channels in xs

    with tile.TileContext(nc) as tc:
        with ExitStack() as ctx:
            wpool = ctx.enter_context(tc.tile_pool(name="wpool", bufs=1))
            ring = ctx.enter_context(tc.tile_pool(name="ring", bufs=RING))
            psum = ctx.enter_context(
                tc.tile_pool(name="psum", bufs=8, space="PSUM")
            )
            stage = ctx.enter_context(tc.tile_pool(name="stage", bufs=4))

            # ---- weights + bias (tiny, once)
            w_t = wpool.tile([NTAP, KS * M], f32r)
            for kd in range(KS):
                nc.sync.dma_start(
                    w_t[:, kd * M : (kd + 1) * M], wt.ap()[kd]
                )
            bias_t = wpool.tile([MHALF, 2], f32)
            nc.sync.dma_start(bias_t[:], bias2.ap())

            handles = {}
            loop_ctx = ExitStack()
            if repeat > 1:
                loop_ctx.enter_context(
                    tc.For_i(
                        0,
                        repeat,
                        1,
                        hint_engines=(
                            mybir.EngineType.PE,
                            mybir.EngineType.DVE,
                            mybir.EngineType.Activation,
                            mybir.EngineType.SP,
                            mybir.EngineType.Pool,
                        ),
                    )
                )

            def load_plane(s):
                t = ring.tile([NTAP, SLOT], f32r, tag="ring", name=f"ring{s}")
                for kh in range(KS):
                    src = bass.AP(
                        xs_ap.tensor,
                        s * PLANE + kh * PH,
                        [[1, KS], [ci_stride, CIN], [1, VALID]],
                    )
                    nc.gpsimd.dma_start(
                        t[kh * 12 : (kh + 1) * 12, 0:VALID], src
                    )
                handles[s] = t

            load_plane(0)
            load_plane(1)

            for d in range(DCHUNK):
                load_plane(d + 2)
                rhs_v = [
                    handles[d + kd][:].rearrange("p (h w) -> p h w", w=PH)
                    for kd in range(KS)
                ]
                for mh in range(2):
                    st = stage.tile([MHALF, HB * NCOL], f32, tag="stage")
                    for hb in range(HB):
                        ps = psum.tile([MHALF, NCOL], f32, tag="ps")
                        for kd in range(KS):
                            lhsT = w_t[
                                :, kd * M + mh * MHALF : kd * M + (mh + 1) * MHALF
                            ]
                            rhs = rhs_v[kd][:, hb * HB : hb * HB + HB, 0:DHW]
                            nc.tensor.matmul(
                                ps[:],
                                lhsT,
                                rhs,
                                start=(kd == 0),
                                stop=(kd == KS - 1),
                            )
                        dst = st[:, hb * NCOL : (hb + 1) * NCOL]
                        if (mh * HB + hb) % 2 == 0:
                            nc.vector.tensor_scalar_add(
                                dst, ps[:], bias_t[:, mh : mh + 1]
                            )
                        else:
                            nc.scalar.activation(
                                dst,
                                ps[:],
                                mybir.ActivationFunctionType.Identity,
                                bias=bias_t[:, mh : mh + 1],
                            )
                    nc.sync.dma_start(
                        y_ap[mh * MHALF : (mh + 1) * MHALF, d], st[:]
                    )

            loop_ctx.close()

    nc.compile()
    return nc


def _make_runner(nc):
    """Build a reusable jitted SPMD executor (no donation so device buffers
    can be reused across timing calls). Modeled on bass2jax.run_bass_via_pjrt."""
    import jax
    import numpy as _np
    from jax.sharding import Mesh, PartitionSpec
    from jax.experimental.shard_map import shard_map

    import concourse.mybir as mybir
    from concourse import bass2jax

    bass2jax.install_neuronx_cc_hook()

    partition_name = (
        nc.partition_id_tensor.name if nc.partition_id_tensor else None
    )
    in_names, out_names, out_avals, zero_outs = [], [], [], []
    for alloc in nc.m.functions[0].allocations:
        if not isinstance(alloc, mybir.MemoryLocationSet):
            continue
        name = alloc.memorylocations[0].name
        if alloc.kind == "ExternalInput":
            if name != partition_name:
                in_names.append(name)
        elif alloc.kind == "ExternalOutput":
            shape = tuple(alloc.tensor_shape)
            dtype = mybir.dt.np(alloc.dtype)
            out_names.append(name)
            out_avals.append(jax.core.ShapedArray(shape, dtype))
            zero_outs.append(_np.zeros(shape, dtype))
    n_params = len(in_names)
    all_names = in_names + out_names
    if partition_name is not None:
        all_names = all_names + [partition_name]

    def _body(*args):
        operands = list(args)
        if partition_name is not None:
            operands.append(bass2jax.partition_id_tensor())
        outs = bass2jax._bass_exec_p.bind(
            *operands,
            out_avals=tuple(out_avals),
            in_names=tuple(all_names),
            out_names=tuple(out_names),
            lowering_input_output_aliases=(),
            sim_require_finite=True,
            sim_require_nnan=True,
            nc=nc,
        )
        return tuple(outs)

    devices = jax.devices()[:N_CORES]
    mesh = Mesh(np.asarray(devices), ("core",))
    n_args = n_params + len(out_names)
    sharded = jax.jit(
        shard_map(
            _body,
            mesh=mesh,
            in_specs=(PartitionSpec("core"),) * n_args,
            out_specs=(PartitionSpec("core"),) * len(out_names),
            check_rep=False,
        ),
        keep_unused=True,
    )

    from jax.sharding import NamedSharding

    shard = NamedSharding(mesh, PartitionSpec("core"))

    def place_inputs(in_maps):
        """Device-put per-core inputs (sharded along axis 0) + cached zero
        output buffers; returns the full arg list, all device-resident."""
        concat = [
            np.concatenate([np.asarray(m[name]) for m in in_maps], axis=0)
            for name in in_names
        ]
        placed = [jax.device_put(a, shard) for a in concat]
        if "zeros" not in _CACHE:
            _CACHE["zeros"] = [
                jax.device_put(
                    np.zeros((N_CORES * z.shape[0], *z.shape[1:]), z.dtype), shard
                )
                for z in zero_outs
            ]
        return placed + _CACHE["zeros"]

    def run(args):
        return sharded(*args)

    return {
        "place_inputs": place_inputs,
        "run": run,
        "out_names": out_names,
        "out_avals": out_avals,
    }


def _get_runner():
    if "runner" not in _CACHE:
        nc = _build_program()
        _CACHE["runner"] = _make_runner(nc)
    return _CACHE["runner"]


def _get_timing_runner(repeat):
    key = f"runner_r{repeat}"
    if key not in _CACHE:
        nc = _build_program(repeat=repeat)
        _CACHE[key] = _make_runner(nc)
    return _CACHE[key]


def _host_prep(x, weight, bias):
    idx = _rot_index_maps(KS)
    wr = weight[:, :, idx[:, 0], idx[:, 1], idx[:, 2]]  # (8,4,24,3,3,3)
    wr = np.transpose(wr, (2, 0, 1, 3, 4, 5)).reshape(M, CIN, KS, KS, KS)
    # lhsT rows ordered (kh, kw, ci) to match the im2col partition order.
    wt = np.ascontiguousarray(
        wr.transpose(2, 3, 4, 1, 0).reshape(KS, NTAP, M), dtype=np.float32
    )
    bias192 = np.broadcast_to(bias[None, :], (N_ROT, COUT)).reshape(M)
    bias2 = np.ascontiguousarray(bias192.reshape(2, MHALF).T, dtype=np.float32)

    x_pad = np.zeros((2, CIN, PH, PH, PH), dtype=np.float32)
    x_pad[:, :, 1:65, 1:65, 1:65] = x

    in_maps = []
    for core in range(N_CORES):
        n, dc = divmod(core, N_CORES // 2)
        slab = np.ascontiguousarray(
            x_pad[n, :, DCHUNK * dc : DCHUNK * dc + SLAB_D]
        )
        in_maps.append({"xs": slab, "wt": wt, "bias2": bias2})
    return in_maps


def kernel(x, weight, bias):
    x = np.asarray(x, dtype=np.float32)
    weight = np.asarray(weight, dtype=np.float32)
    bias = np.asarray(bias, dtype=np.float32)

    runner = _get_runner()
    in_maps = _host_prep(x, weight, bias)
    args = runner["place_inputs"](in_maps)
    out = runner["run"](args)
    y8 = np.asarray(out[0]).reshape(N_CORES, M, DCHUNK, DHW, DHW)

    yfull = np.empty((2, M, DHW, DHW, DHW), dtype=np.float32)
    for core in range(N_CORES):
        n, dc = divmod(core, N_CORES // 2)
        yfull[n, :, DCHUNK * dc : DCHUNK * (dc + 1)] = y8[core]
    return yfull.reshape(2, N_ROT, COUT, DHW, DHW, DHW)
